# revision 5
# baseline (speedup 1.0000x reference)
"""MEGNet layer kernel for 8 Trainium2 NeuronCores.

Strategy (graph/data parallel per sharding hint):
  - Bonds (800k) and atoms (100k) are sharded across 8 cores on the leading dim.
  - The tiny MLP weights are folded/replicated host-side:
      * the global-feature columns of each first-layer weight are folded into
        an effective bias (global state is a single broadcast row),
      * the duplicated atom_features block in the atom MLP input is folded by
        summing the two weight blocks,
      * the 1/deg segment-mean scale is applied host-side.
  - The neighbor gathers (atom endpoints per bond; segment-mean of updated
    bonds per atom) are index-only transforms of kernel inputs/phase-1 output,
    performed host-side while assembling per-core shards; each core then runs
    a pure streaming 2-layer MLP (matmul -> softplus -> matmul -> residual)
    with activations kept feature-major ([64|128, N] tiles) so the tensor
    engine contracts along partitions with zero on-chip transposes.
  - Phase 2 (atom update) depends on all of phase 1's bond output, so the
    kernel runs as two SPMD launches with a host gather between them.

Softplus is composed as ln(1+exp(x)) on the scalar engine (this build's
activation tables have exp/ln in one table but no native softplus).
"""

import sys

sys.path.insert(0, "/opt/trn_rl_repo")

import numpy as np

from concourse import bass, bacc, mybir, tile
from concourse.bass_utils import run_bass_kernel_spmd

F32 = mybir.dt.float32

N_ATOMS, N_BONDS, MAX_DEG, F = 100000, 800000, 32, 64
HID = 128
N_CORES = 8
BONDS_PER_CORE = N_BONDS // N_CORES   # 100000
ATOMS_PER_CORE = N_ATOMS // N_CORES   # 12500
NT = 512                              # moving-dim tile (fp32 max, 1 PSUM bank)


def build_mlp_program(n_cols, k2, name):
    """Streaming 2-layer MLP over n_cols columns.

    Inputs (feature-major):
      x1   [128, n_cols]  first contraction chunk (K=128)
      x2   [k2,  n_cols]  second contraction chunk (K=k2, optional residual src)
      res  picked from x2 (phase 1: bondT) or x1[0:64] (phase 2: atomT)
      w1a [128,HID], w1b [k2,HID] or None, b1 [HID,1], w2 [HID,F], b2 [F,1]
    Output: out [F, n_cols] = w2.T @ softplus(w1.T @ x + b1) + b2 + res
    """
    nc = bacc.Bacc("TRN2", target_bir_lowering=False, debug=False,
                   num_devices=N_CORES)
    x1 = nc.dram_tensor("x1", [128, n_cols], F32, kind="ExternalInput")
    have_x2 = k2 > 0
    if have_x2:
        x2 = nc.dram_tensor("x2", [k2, n_cols], F32, kind="ExternalInput")
    w1a_d = nc.dram_tensor("w1a", [128, HID], F32, kind="ExternalInput")
    if have_x2:
        w1b_d = nc.dram_tensor("w1b", [k2, HID], F32, kind="ExternalInput")
    b1_d = nc.dram_tensor("b1", [HID, 1], F32, kind="ExternalInput")
    w2_d = nc.dram_tensor("w2", [HID, F], F32, kind="ExternalInput")
    out = nc.dram_tensor("out", [F, n_cols], F32, kind="ExternalOutput")

    n_tiles = (n_cols + NT - 1) // NT
    with tile.TileContext(nc) as tc:
        with tc.tile_pool(name="wpool", bufs=1) as wp, \
             tc.tile_pool(name="io", bufs=3) as io, \
             tc.tile_pool(name="mid", bufs=3) as mid, \
             tc.tile_pool(name="ps", bufs=2, space="PSUM") as ps:
            w1a = wp.tile([128, HID], F32, tag="w1a")
            nc.sync.dma_start(out=w1a[:], in_=w1a_d[:, :])
            if have_x2:
                w1b = wp.tile([k2, HID], F32, tag="w1b")
                nc.sync.dma_start(out=w1b[:], in_=w1b_d[:, :])
            b1 = wp.tile([HID, 1], F32, tag="b1")
            nc.sync.dma_start(out=b1[:], in_=b1_d[:, :])
            w2 = wp.tile([HID, F], F32, tag="w2")
            nc.sync.dma_start(out=w2[:], in_=w2_d[:, :])

            for t in range(n_tiles):
                c0 = t * NT
                w = min(NT, n_cols - c0)
                x1t = io.tile([128, NT], F32, tag="x1t")
                nc.sync.dma_start(out=x1t[:, :w], in_=x1[:, c0:c0 + w])
                if have_x2:
                    x2t = io.tile([k2, NT], F32, tag="x2t")
                    nc.sync.dma_start(out=x2t[:, :w], in_=x2[:, c0:c0 + w])
                hp = ps.tile([HID, NT], F32, tag="hp")
                nc.tensor.matmul(out=hp[:, :w], lhsT=w1a[:], rhs=x1t[:, :w],
                                 start=True, stop=not have_x2)
                if have_x2:
                    nc.tensor.matmul(out=hp[:, :w], lhsT=w1b[:], rhs=x2t[:, :w],
                                     start=False, stop=True)
                # softplus(h + b1) = ln(1 + exp(h + b1))
                # (activation issued in <=256-wide chunks: walrus lower_act
                # fails to pick a table for 512-wide activations)
                hs = mid.tile([HID, NT], F32, tag="hs")
                for a0 in range(0, w, 256):
                    aw = min(256, w - a0)
                    nc.scalar.activation(out=hs[:, a0:a0 + aw],
                                         in_=hp[:, a0:a0 + aw],
                                         func=mybir.ActivationFunctionType.Exp,
                                         bias=b1[:])
                for a0 in range(0, w, 256):
                    aw = min(256, w - a0)
                    nc.scalar.activation(out=hs[:, a0:a0 + aw],
                                         in_=hs[:, a0:a0 + aw],
                                         func=mybir.ActivationFunctionType.Ln,
                                         bias=1.0)
                op = ps.tile([F, NT], F32, tag="op")
                nc.tensor.matmul(out=op[:, :w], lhsT=w2[:], rhs=hs[:, :w],
                                 start=True, stop=True)
                ot = io.tile([F, NT], F32, tag="ot")
                # out = op + residual (b2 is added host-side during assembly)
                res = x2t[:, :w] if have_x2 else x1t[0:F, :w]
                nc.vector.tensor_add(out=ot[:, :w], in0=op[:, :w], in1=res)
                nc.sync.dma_start(out=out[:, c0:c0 + w], in_=ot[:, :w])
    nc.compile()
    return nc


_PROGRAM_CACHE = {}


def _get_program(n_cols, k2, name):
    key = (n_cols, k2)
    if key not in _PROGRAM_CACHE:
        _PROGRAM_CACHE[key] = build_mlp_program(n_cols, k2, name)
    return _PROGRAM_CACHE[key]


def _run_phase(nc, in_maps):
    res = run_bass_kernel_spmd(nc, in_maps, core_ids=list(range(N_CORES)))
    return [r["out"] for r in res.results]


def _softplus_np(x):
    return np.logaddexp(np.float32(0.0), x)


def kernel(atom_features, bond_features, global_features,
           Wb1, bb1, Wb2, bb2, Wa1, ba1, Wa2, ba2, Wg1, bg1, Wg2, bg2,
           atom_bond_indices, bond_atom_indices):
    # accept jax or numpy inputs
    af = np.ascontiguousarray(np.asarray(atom_features), dtype=np.float32)
    bf = np.ascontiguousarray(np.asarray(bond_features), dtype=np.float32)
    g = np.asarray(np.asarray(global_features), dtype=np.float32)
    Wb1, bb1, Wb2, bb2 = (np.asarray(x) for x in (Wb1, bb1, Wb2, bb2))
    Wa1, ba1, Wa2, ba2 = (np.asarray(x) for x in (Wa1, ba1, Wa2, ba2))
    Wg1, bg1, Wg2, bg2 = (np.asarray(x) for x in (Wg1, bg1, Wg2, bg2))
    atom_bond_indices = np.asarray(atom_bond_indices)
    bond_atom_indices = np.asarray(bond_atom_indices)

    # ---- phase 1: bond update --------------------------------------------
    # comb_b = [atom_i, atom_j, bond, g]; fold g into the bias.
    b1_eff = (bb1 + g[0] @ Wb1[3 * F:4 * F]).astype(np.float32).reshape(HID, 1)
    w1a = np.ascontiguousarray(Wb1[:2 * F], dtype=np.float32)        # [128,HID]
    w1b = np.ascontiguousarray(Wb1[2 * F:3 * F], dtype=np.float32)   # [64,HID]
    w2 = np.ascontiguousarray(Wb2, dtype=np.float32)

    ai = af[atom_bond_indices[:, 0]]          # [Nb, F] host gather
    aj = af[atom_bond_indices[:, 1]]

    nc1 = _get_program(BONDS_PER_CORE, F, "bond")
    in_maps = []
    for c in range(N_CORES):
        sl = slice(c * BONDS_PER_CORE, (c + 1) * BONDS_PER_CORE)
        x1 = np.empty((128, BONDS_PER_CORE), np.float32)
        x1[:F] = ai[sl].T
        x1[F:] = aj[sl].T
        in_maps.append({
            "x1": x1,
            "x2": np.ascontiguousarray(bf[sl].T),
            "w1a": w1a, "w1b": w1b, "b1": b1_eff, "w2": w2,
        })
    outs = _run_phase(nc1, in_maps)
    updated_bonds = np.hstack(outs).T + np.asarray(bb2, np.float32)  # [Nb, F]
    updated_bonds = np.ascontiguousarray(updated_bonds, dtype=np.float32)

    # ---- segment mean over padded neighbor lists (host gather) -----------
    agg = np.empty((N_ATOMS, F), np.float32)
    CH = 20000
    for s in range(0, N_ATOMS, CH):
        idx = bond_atom_indices[s:s + CH]
        m = idx >= 0
        vals = updated_bonds[np.where(m, idx, 0)] * m[..., None]
        cnt = m.sum(axis=1, keepdims=True).astype(np.float32)
        agg[s:s + CH] = vals.sum(axis=1) / np.maximum(cnt, 1.0)

    # ---- phase 2: atom update --------------------------------------------
    # comb_a = [atom, agg, atom, g]; fold dup atom block + g bias.
    b1a_eff = (ba1 + g[0] @ Wa1[3 * F:4 * F]).astype(np.float32).reshape(HID, 1)
    w1a_at = np.ascontiguousarray(
        np.vstack([Wa1[:F] + Wa1[2 * F:3 * F], Wa1[F:2 * F]]), dtype=np.float32)
    w2a = np.ascontiguousarray(Wa2, dtype=np.float32)

    nc2 = _get_program(ATOMS_PER_CORE, 0, "atom")
    in_maps = []
    for c in range(N_CORES):
        sl = slice(c * ATOMS_PER_CORE, (c + 1) * ATOMS_PER_CORE)
        x1 = np.empty((128, ATOMS_PER_CORE), np.float32)
        x1[:F] = af[sl].T
        x1[F:] = agg[sl].T
        in_maps.append({
            "x1": x1,
            "w1a": w1a_at, "b1": b1a_eff, "w2": w2a,
        })
    outs = _run_phase(nc2, in_maps)
    updated_atoms = np.hstack(outs).T + np.asarray(ba2, np.float32)  # [Na, F]
    updated_atoms = np.ascontiguousarray(updated_atoms, dtype=np.float32)

    # ---- global update (tiny: [1,192] @ [192,128] @ [128,64]) ------------
    comb_g = np.concatenate([updated_atoms.mean(axis=0, keepdims=True),
                             updated_bonds.mean(axis=0, keepdims=True),
                             g], axis=-1).astype(np.float32)
    hg = _softplus_np(comb_g @ Wg1 + bg1)
    updated_global = (hg @ Wg2 + bg2 + g).astype(np.float32)

    return updated_atoms, updated_bonds, updated_global


# revision 6
# speedup vs baseline: 1.1978x; 1.1978x over previous
"""MEGNet layer kernel for 8 Trainium2 NeuronCores.

Strategy (graph/data parallel per sharding hint):
  - Bonds (800k) and atoms (100k) are sharded across 8 cores on the leading dim.
  - The tiny MLP weights are folded/replicated host-side:
      * the global-feature columns of each first-layer weight are folded into
        an effective bias (global state is a single broadcast row),
      * the duplicated atom_features block in the atom MLP input is folded by
        summing the two weight blocks,
      * the 1/deg segment-mean scale is applied host-side.
  - The neighbor gathers (atom endpoints per bond; segment-mean of updated
    bonds per atom) are index-only transforms of kernel inputs/phase-1 output,
    performed host-side while assembling per-core shards; each core then runs
    a pure streaming 2-layer MLP (matmul -> softplus -> matmul -> residual)
    with activations kept feature-major ([64|128, N] tiles) so the tensor
    engine contracts along partitions with zero on-chip transposes.
  - Phase 2 (atom update) depends on all of phase 1's bond output, so the
    kernel runs as two SPMD launches with a host gather between them.

Softplus is composed as ln(1+exp(x)) on the scalar engine (this build's
activation tables have exp/ln in one table but no native softplus).
"""

import sys

sys.path.insert(0, "/opt/trn_rl_repo")

import ml_dtypes
import numpy as np

from concourse import bass, bacc, mybir, tile
from concourse.bass_utils import run_bass_kernel_spmd

F32 = mybir.dt.float32
BF16 = mybir.dt.bfloat16
NP_BF16 = ml_dtypes.bfloat16

N_ATOMS, N_BONDS, MAX_DEG, F = 100000, 800000, 32, 64
HID = 128
N_CORES = 8
BONDS_PER_CORE = N_BONDS // N_CORES   # 100000
ATOMS_PER_CORE = N_ATOMS // N_CORES   # 12500
NT = 512                              # moving-dim tile (fp32 max, 1 PSUM bank)


def build_mlp_program(n_cols, k2, name):
    """Streaming 2-layer MLP over n_cols columns.

    Inputs (feature-major):
      x1   [128, n_cols]  first contraction chunk (K=128)
      x2   [k2,  n_cols]  second contraction chunk (K=k2, optional residual src)
      res  picked from x2 (phase 1: bondT) or x1[0:64] (phase 2: atomT)
      w1a [128,HID], w1b [k2,HID] or None, b1 [HID,1], w2 [HID,F], b2 [F,1]
    Output: out [F, n_cols] = w2.T @ softplus(w1.T @ x + b1) + b2 + res
    """
    nc = bacc.Bacc("TRN2", target_bir_lowering=False, debug=False,
                   num_devices=N_CORES)
    have_x2 = k2 > 0
    # phase 1 (have_x2): bf16 matmul path, 4x PE throughput + half the x1
    # upload; the residual source x2 stays fp32 so the skip-connection that
    # dominates the output is exact. phase 2 stays all-fp32 (it is tiny).
    MD = BF16 if have_x2 else F32
    x1 = nc.dram_tensor("x1", [128, n_cols], MD, kind="ExternalInput")
    if have_x2:
        x2 = nc.dram_tensor("x2", [k2, n_cols], F32, kind="ExternalInput")
    w1a_d = nc.dram_tensor("w1a", [128, HID], MD, kind="ExternalInput")
    if have_x2:
        w1b_d = nc.dram_tensor("w1b", [k2, HID], MD, kind="ExternalInput")
    b1_d = nc.dram_tensor("b1", [HID, 1], F32, kind="ExternalInput")
    w2_d = nc.dram_tensor("w2", [HID, F], MD, kind="ExternalInput")
    out = nc.dram_tensor("out", [F, n_cols], F32, kind="ExternalOutput")

    n_tiles = (n_cols + NT - 1) // NT
    with tile.TileContext(nc) as tc:
        with tc.tile_pool(name="wpool", bufs=1) as wp, \
             tc.tile_pool(name="io", bufs=3) as io, \
             tc.tile_pool(name="mid", bufs=3) as mid, \
             tc.tile_pool(name="ps", bufs=2, space="PSUM") as ps:
            w1a = wp.tile([128, HID], MD, tag="w1a")
            nc.sync.dma_start(out=w1a[:], in_=w1a_d[:, :])
            if have_x2:
                w1b = wp.tile([k2, HID], MD, tag="w1b")
                nc.sync.dma_start(out=w1b[:], in_=w1b_d[:, :])
            b1 = wp.tile([HID, 1], F32, tag="b1")
            nc.sync.dma_start(out=b1[:], in_=b1_d[:, :])
            w2 = wp.tile([HID, F], MD, tag="w2")
            nc.sync.dma_start(out=w2[:], in_=w2_d[:, :])

            for t in range(n_tiles):
                c0 = t * NT
                w = min(NT, n_cols - c0)
                x1t = io.tile([128, NT], MD, tag="x1t")
                nc.sync.dma_start(out=x1t[:, :w], in_=x1[:, c0:c0 + w])
                if have_x2:
                    x2t = io.tile([k2, NT], F32, tag="x2t")
                    nc.sync.dma_start(out=x2t[:, :w], in_=x2[:, c0:c0 + w])
                    x2b = mid.tile([k2, NT], BF16, tag="x2b")
                    nc.vector.tensor_copy(out=x2b[:, :w], in_=x2t[:, :w])
                hp = ps.tile([HID, NT], F32, tag="hp")
                nc.tensor.matmul(out=hp[:, :w], lhsT=w1a[:], rhs=x1t[:, :w],
                                 start=True, stop=not have_x2)
                if have_x2:
                    nc.tensor.matmul(out=hp[:, :w], lhsT=w1b[:], rhs=x2b[:, :w],
                                     start=False, stop=True)
                # softplus(h + b1) = ln(1 + exp(h + b1))
                # (activation issued in <=256-wide chunks: walrus lower_act
                # fails to pick a table for 512-wide activations)
                hs = mid.tile([HID, NT], MD, tag="hs")
                for a0 in range(0, w, 256):
                    aw = min(256, w - a0)
                    nc.scalar.activation(out=hs[:, a0:a0 + aw],
                                         in_=hp[:, a0:a0 + aw],
                                         func=mybir.ActivationFunctionType.Exp,
                                         bias=b1[:])
                for a0 in range(0, w, 256):
                    aw = min(256, w - a0)
                    nc.scalar.activation(out=hs[:, a0:a0 + aw],
                                         in_=hs[:, a0:a0 + aw],
                                         func=mybir.ActivationFunctionType.Ln,
                                         bias=1.0)
                op = ps.tile([F, NT], F32, tag="op")
                nc.tensor.matmul(out=op[:, :w], lhsT=w2[:], rhs=hs[:, :w],
                                 start=True, stop=True)
                ot = io.tile([F, NT], F32, tag="ot")
                # out = op + residual (b2 is added host-side during assembly)
                res = x2t[:, :w] if have_x2 else x1t[0:F, :w]
                nc.vector.tensor_add(out=ot[:, :w], in0=op[:, :w], in1=res)
                nc.sync.dma_start(out=out[:, c0:c0 + w], in_=ot[:, :w])
    nc.compile()
    return nc


_PROGRAM_CACHE = {}


def _get_program(n_cols, k2, name):
    key = (n_cols, k2)
    if key not in _PROGRAM_CACHE:
        _PROGRAM_CACHE[key] = build_mlp_program(n_cols, k2, name)
    return _PROGRAM_CACHE[key]


def _run_phase(nc, in_maps):
    res = run_bass_kernel_spmd(nc, in_maps, core_ids=list(range(N_CORES)))
    return [r["out"] for r in res.results]


def _softplus_np(x):
    return np.logaddexp(np.float32(0.0), x)


def kernel(atom_features, bond_features, global_features,
           Wb1, bb1, Wb2, bb2, Wa1, ba1, Wa2, ba2, Wg1, bg1, Wg2, bg2,
           atom_bond_indices, bond_atom_indices):
    # accept jax or numpy inputs
    af = np.ascontiguousarray(np.asarray(atom_features), dtype=np.float32)
    bf = np.ascontiguousarray(np.asarray(bond_features), dtype=np.float32)
    g = np.asarray(np.asarray(global_features), dtype=np.float32)
    Wb1, bb1, Wb2, bb2 = (np.asarray(x) for x in (Wb1, bb1, Wb2, bb2))
    Wa1, ba1, Wa2, ba2 = (np.asarray(x) for x in (Wa1, ba1, Wa2, ba2))
    Wg1, bg1, Wg2, bg2 = (np.asarray(x) for x in (Wg1, bg1, Wg2, bg2))
    atom_bond_indices = np.asarray(atom_bond_indices)
    bond_atom_indices = np.asarray(bond_atom_indices)

    # ---- phase 1: bond update --------------------------------------------
    # comb_b = [atom_i, atom_j, bond, g]; fold g into the bias.
    b1_eff = (bb1 + g[0] @ Wb1[3 * F:4 * F]).astype(np.float32).reshape(HID, 1)
    w1a = np.ascontiguousarray(Wb1[:2 * F], dtype=NP_BF16)           # [128,HID]
    w1b = np.ascontiguousarray(Wb1[2 * F:3 * F], dtype=NP_BF16)      # [64,HID]
    w2 = np.ascontiguousarray(Wb2, dtype=NP_BF16)

    ai = af[atom_bond_indices[:, 0]]          # [Nb, F] host gather
    aj = af[atom_bond_indices[:, 1]]

    nc1 = _get_program(BONDS_PER_CORE, F, "bond")
    in_maps = []
    for c in range(N_CORES):
        sl = slice(c * BONDS_PER_CORE, (c + 1) * BONDS_PER_CORE)
        x1 = np.empty((128, BONDS_PER_CORE), NP_BF16)
        x1[:F] = ai[sl].T.astype(NP_BF16)
        x1[F:] = aj[sl].T.astype(NP_BF16)
        in_maps.append({
            "x1": x1,
            "x2": np.ascontiguousarray(bf[sl].T),
            "w1a": w1a, "w1b": w1b, "b1": b1_eff, "w2": w2,
        })
    outs = _run_phase(nc1, in_maps)
    updated_bonds = np.hstack(outs).T + np.asarray(bb2, np.float32)  # [Nb, F]
    updated_bonds = np.ascontiguousarray(updated_bonds, dtype=np.float32)

    # ---- segment mean over padded neighbor lists (host gather) -----------
    agg = np.empty((N_ATOMS, F), np.float32)
    CH = 20000
    for s in range(0, N_ATOMS, CH):
        idx = bond_atom_indices[s:s + CH]
        m = idx >= 0
        vals = updated_bonds[np.where(m, idx, 0)] * m[..., None]
        cnt = m.sum(axis=1, keepdims=True).astype(np.float32)
        agg[s:s + CH] = vals.sum(axis=1) / np.maximum(cnt, 1.0)

    # ---- phase 2: atom update --------------------------------------------
    # comb_a = [atom, agg, atom, g]; fold dup atom block + g bias.
    b1a_eff = (ba1 + g[0] @ Wa1[3 * F:4 * F]).astype(np.float32).reshape(HID, 1)
    w1a_at = np.ascontiguousarray(
        np.vstack([Wa1[:F] + Wa1[2 * F:3 * F], Wa1[F:2 * F]]), dtype=np.float32)
    w2a = np.ascontiguousarray(Wa2, dtype=np.float32)

    nc2 = _get_program(ATOMS_PER_CORE, 0, "atom")
    in_maps = []
    for c in range(N_CORES):
        sl = slice(c * ATOMS_PER_CORE, (c + 1) * ATOMS_PER_CORE)
        x1 = np.empty((128, ATOMS_PER_CORE), np.float32)
        x1[:F] = af[sl].T
        x1[F:] = agg[sl].T
        in_maps.append({
            "x1": x1,
            "w1a": w1a_at, "b1": b1a_eff, "w2": w2a,
        })
    outs = _run_phase(nc2, in_maps)
    updated_atoms = np.hstack(outs).T + np.asarray(ba2, np.float32)  # [Na, F]
    updated_atoms = np.ascontiguousarray(updated_atoms, dtype=np.float32)

    # ---- global update (tiny: [1,192] @ [192,128] @ [128,64]) ------------
    comb_g = np.concatenate([updated_atoms.mean(axis=0, keepdims=True),
                             updated_bonds.mean(axis=0, keepdims=True),
                             g], axis=-1).astype(np.float32)
    hg = _softplus_np(comb_g @ Wg1 + bg1)
    updated_global = (hg @ Wg2 + bg2 + g).astype(np.float32)

    return updated_atoms, updated_bonds, updated_global
